# revision 7
# baseline (speedup 1.0000x reference)
"""DSMIL forward pass on 8 Trainium2 NeuronCores (Bass/Tile), bf16 compute.

Sharding: data-parallel over bags, each bag split across a core pair
(core 2b: instances [0:4096) of bag b, core 2b+1: [4096:8192)). Two tiny
pair-local collectives (critical-instance exchange + softmax partial
reduction) keep it a single NEFF launch.

Key implementation choices (vs the fp32r baseline):
  - x is transposed and cast to bf16 on the HOST: the kernel streams
    xT i-block tiles straight into matmuls (no on-chip x transposes,
    half the HBM traffic).
  - all matmuls run in bf16 (fp32 PSUM accumulation); the class-score
    path stays fp32 from PSUM onward so the per-class argmax is stable.
  - classes are computed in NATURAL [n, c] layout directly
    (lhsT = h^T block, rhs = W_i block), so the per-class max reduction
    runs on all 128 DVE lanes instead of 2.
  - weights/constants load via one consolidated DMA each, spread over
    the gpsimd/scalar/vector queues; x chunks own the sync queue.
  - dummy PE transposes at kernel start pre-warm the HAM clock gate
    while the first DMAs land; another batch keeps it warm through the
    AllGather wait.
  - the pair exchanges (m_feats, max) and computes q_fn(m) AFTER the
    winner select; the Q = q_fn(h) pass is emitted around the exchange
    so the collective latency is hidden under real work.
"""
import numpy as np
import ml_dtypes
from contextlib import ExitStack

import concourse.bacc as bacc
import concourse.tile as tile
import concourse.mybir as mybir

F32 = mybir.dt.float32
BF16 = mybir.dt.bfloat16
AF = mybir.ActivationFunctionType
ALU = mybir.AluOpType
bfdt = ml_dtypes.bfloat16

N_CORES = 8
B_BAGS = 4
N_FULL = 8192
N_LOC = N_FULL // 2

_cache = {}


def _build_kernel(n_cores=N_CORES, N_loc=N_LOC, I=1024, D=512, QD=128,
                  C=2, CHUNK=512, N_WARM0=30, N_WARM=64):
    NB = N_loc // 128          # n-blocks (32)
    NCH = N_loc // CHUNK       # chunks (8)
    BPC = CHUNK // 128         # n-blocks per chunk (4)
    IB = I // 128              # i-blocks (8)
    DB = D // 128              # d-blocks (4)
    assert QD == 128 and C == 2
    inv_sqrt_q = 1.0 / float(np.sqrt(QD))

    nc = bacc.Bacc("TRN2", target_bir_lowering=False, debug=False,
                   num_devices=n_cores)

    xt_d = nc.dram_tensor("xt", [I, N_loc], BF16, kind="ExternalInput")
    w_enc = nc.dram_tensor("w_enc", [128, IB, D], BF16, kind="ExternalInput")
    w_i = nc.dram_tensor("w_i", [128, DB, C], BF16, kind="ExternalInput")
    w_q1 = nc.dram_tensor("w_q1", [128, DB, QD], BF16, kind="ExternalInput")
    w_q2 = nc.dram_tensor("w_q2", [QD, QD], BF16, kind="ExternalInput")
    bias_d = nc.dram_tensor("bias", [128, DB + 2], F32, kind="ExternalInput")
    b_i_d = nc.dram_tensor("b_i", [1, C], BF16, kind="ExternalInput")
    identb_d = nc.dram_tensor("identb", [128, 128], BF16,
                              kind="ExternalInput")
    identf_d = nc.dram_tensor("identf", [128, 128], F32,
                              kind="ExternalInput")
    out_d = nc.dram_tensor("out", [C, D], F32, kind="ExternalOutput")

    groups = [[i, i + 1] for i in range(0, n_cores, 2)]

    with tile.TileContext(nc) as tc, ExitStack() as ctx:
        persist = ctx.enter_context(tc.tile_pool(name="persist", bufs=1))
        dram = ctx.enter_context(tc.tile_pool(name="dram", bufs=1,
                                              space="DRAM"))

        # ---- scratch consts on the (idle) vector queue ----
        scrap = persist.tile([128, 128], BF16)
        nc.vector.memset(scrap[:], 0.0)
        ones_t = persist.tile([1, 128], F32)
        nc.vector.memset(ones_t[:], 1.0)
        ones_bf = persist.tile([1, 128], BF16)
        nc.vector.memset(ones_bf[:], 1.0)
        warm_in = dram.tile([1, 2], F32)
        nc.scalar.dma_start(warm_in[:], identf_d[0:1, 0:2])
        warm_in2 = dram.tile([1, 2], F32)
        nc.scalar.dma_start(warm_in2[:], identf_d[0:1, 0:2])

        # ---- consolidated weight loads (gpsimd queue) ----
        w_enc_sb = persist.tile([128, IB, D], BF16)
        nc.gpsimd.dma_start(w_enc_sb[:], w_enc[:])
        w_q1_sb = persist.tile([128, DB, QD], BF16)
        nc.gpsimd.dma_start(w_q1_sb[:], w_q1[:])
        w_i_sb = persist.tile([128, DB, C], BF16)
        nc.gpsimd.dma_start(w_i_sb[:], w_i[:])
        w_q2_sb = persist.tile([QD, QD], BF16)
        nc.gpsimd.dma_start(w_q2_sb[:], w_q2[:])

        # warm both collective channels (fires once weights are queued)
        warm_out = dram.tile([2, 2], F32)
        nc.gpsimd.collective_compute(
            "AllGather", ALU.bypass, replica_groups=groups,
            ins=[warm_in[:].opt()], outs=[warm_out[:].opt()])
        warm_out2 = dram.tile([1, 2], F32)
        nc.gpsimd.collective_compute(
            "AllReduce", ALU.add, replica_groups=groups,
            ins=[warm_in2[:].opt()], outs=[warm_out2[:].opt()])

        # ---- small consts (scalar queue) ----
        identb = persist.tile([128, 128], BF16)
        nc.scalar.dma_start(identb[:], identb_d[:])
        identf = persist.tile([128, 128], F32)
        nc.scalar.dma_start(identf[:], identf_d[:])
        bias_sb = persist.tile([128, DB + 2], F32)
        nc.scalar.dma_start(bias_sb[:], bias_d[:])
        b_i_bf = persist.tile([1, C], BF16)
        nc.scalar.dma_start(b_i_bf[:], b_i_d[:])

        # ---- persistent activations ----
        ht_all = persist.tile([128, NCH, DB, CHUNK], BF16)   # h^T
        h_nat = persist.tile([128, NB, D], BF16)             # h natural
        qt = persist.tile([128, NCH, CHUNK], BF16)           # Q^T
        cls_nat = persist.tile([128, NB, C], F32)
        oh = persist.tile([128, NB, C], BF16)
        e_nat = persist.tile([128, NB, C], BF16)

        # ================= phase A: encoder + classes + h transposes ====
        with (
            tc.tile_pool(name="xload", bufs=2) as xload,
            tc.tile_pool(name="hp", bufs=2, space="PSUM") as hp,
            tc.tile_pool(name="tp", bufs=1, space="PSUM") as tp,
            tc.tile_pool(name="cp", bufs=2, space="PSUM") as cp,
        ):
            # pre-warm the PE clock gate while the first DMAs land
            pw = tp.tile([128, D], BF16, tag="t0", name="t")
            for k in range(N_WARM0):
                nc.tensor.transpose(pw[:, 0:128], scrap[:], scrap[:])

            for cb in range(NCH):
                n0 = cb * CHUNK
                xt_c = xload.tile([128, IB, CHUNK], BF16, tag="x", name="x")
                src = xt_d[:, n0:n0 + CHUNK].rearrange("(ib p) n -> p ib n",
                                                       p=128)
                nc.sync.dma_start(xt_c[:], src)

                # h^T = relu(W_enc^T @ xT) per d-block
                for db in range(DB):
                    ph = hp.tile([128, CHUNK], F32, tag="h", name="h")
                    for ib in range(IB):
                        nc.tensor.matmul(
                            ph[:],
                            w_enc_sb[:, ib, db * 128:(db + 1) * 128],
                            xt_c[:, ib, :],
                            start=(ib == 0), stop=(ib == IB - 1))
                    nc.scalar.activation(ht_all[:, cb, db, :], ph[:],
                                         AF.Relu,
                                         bias=bias_sb[:, db:db + 1])

                # h natural layout: 4 transposes per n-block into one tile
                for b in range(BPC):
                    nb = cb * BPC + b
                    pt = tp.tile([128, D], BF16, tag=f"t{b}", name="t")
                    for db in range(DB):
                        nc.tensor.transpose(
                            pt[:, db * 128:(db + 1) * 128],
                            ht_all[:, cb, db, b * 128:(b + 1) * 128],
                            identb[:])
                    nc.vector.tensor_copy(h_nat[:, nb, :], pt[:])

                # classes in natural layout: lhsT = h^T block, rhs = W_i
                pc = cp.tile([128, BPC, C], F32, tag="c", name="c")
                for b in range(BPC):
                    for db in range(DB):
                        nc.tensor.matmul(
                            pc[:, b, :],
                            ht_all[:, cb, db, b * 128:(b + 1) * 128],
                            w_i_sb[:, db, :],
                            start=(db == 0), stop=False)
                    nc.tensor.matmul(pc[:, b, :], ones_bf[:], b_i_bf[:],
                                     start=False, stop=True)
                nc.vector.tensor_copy(
                    cls_nat[:, cb * BPC:(cb + 1) * BPC, :], pc[:])

        # ====== exchange prep + Q-pass (hides the AllGather) ===========
        with (
            tc.tile_pool(name="zp", bufs=2, space="PSUM") as zp,
            tc.tile_pool(name="qp", bufs=1, space="PSUM") as qp,
            tc.tile_pool(name="pa", bufs=1, space="PSUM") as pa,
            tc.tile_pool(name="zs", bufs=2) as zs,
        ):
            def q_pass(cb):
                pz = zp.tile([128, CHUNK], F32, tag="z", name="z")
                for db in range(DB):
                    nc.tensor.matmul(pz[:], w_q1_sb[:, db, :],
                                     ht_all[:, cb, db, :],
                                     start=(db == 0), stop=(db == DB - 1))
                zt = zs.tile([128, CHUNK], BF16, tag="zt", name="zt")
                nc.vector.tensor_scalar(zt[:], pz[:],
                                        bias_sb[:, DB:DB + 1], 0.0,
                                        ALU.add, ALU.max)
                pq = qp.tile([128, CHUNK], F32, tag="q", name="q")
                nc.tensor.matmul(pq[:], w_q2_sb[:], zt[:], start=True,
                                 stop=True)
                nc.scalar.activation(qt[:, cb, :], pq[:], AF.Tanh,
                                     bias=bias_sb[:, DB + 1:DB + 2])

            # per-class max across all instances
            rmax = persist.tile([128, C], F32)
            nc.vector.reduce_max(rmax[:],
                                 cls_nat[:].rearrange("p nb c -> p c nb"),
                                 axis=mybir.AxisListType.X)
            q_pass(0)
            pmv = pa.tile([C, 128], F32, name="pmv")
            nc.tensor.transpose(pmv[:], rmax[:], identf[:])
            mval_c = persist.tile([C, 1], F32)
            nc.vector.reduce_max(mval_c[:], pmv[:],
                                 axis=mybir.AxisListType.X)
            pmr = pa.tile([1, C], F32, name="pmr")
            nc.tensor.transpose(pmr[:], mval_c[:], identf[0:2, 0:2])
            mval_f = persist.tile([1, C], F32)
            nc.vector.tensor_copy(mval_f[:], pmr[:])
            pmb = pa.tile([128, C], F32, name="pmb")
            nc.tensor.matmul(pmb[:], ones_t[:], mval_f[:],
                             start=True, stop=True)
            mb = persist.tile([128, C], F32)
            nc.vector.tensor_copy(mb[:], pmb[:])
            q_pass(1)

            for nb in range(NB):
                nc.vector.tensor_tensor(oh[:, nb, :], cls_nat[:, nb, :],
                                        mb[:], ALU.is_equal)

            # m = onehot^T @ h  (critical instance features)
            pmf = pa.tile([C, D], F32, name="pmf")
            for nb in range(NB):
                nc.tensor.matmul(pmf[:], oh[:, nb, :], h_nat[:, nb, :],
                                 start=(nb == 0), stop=(nb == NB - 1))
            pay_sb = persist.tile([C, D + 1], F32)
            nc.scalar.copy(pay_sb[:, 0:D], pmf[:])
            nc.scalar.copy(pay_sb[:, D:D + 1], mval_c[:])
            pay1 = dram.tile([C, D + 1], F32)
            nc.scalar.dma_start(pay1[:], pay_sb[:])
            gath1 = dram.tile([2 * C, D + 1], F32)
            nc.gpsimd.collective_compute(
                "AllGather", ALU.bypass, replica_groups=groups,
                ins=[pay1[:].opt()], outs=[gath1[:].opt()])

            for cb in range(2, NCH):
                q_pass(cb)

            # keep the PE clock gate warm while waiting on the collective
            pwm = pa.tile([128, 128], BF16, name="pwm")
            for k in range(N_WARM):
                nc.tensor.transpose(pwm[:], identb[:], identb[:])

        # ================= phase B: winner, scores, bag output =========
        with (
            tc.tile_pool(name="pt2", bufs=2, space="PSUM") as pt2,
            tc.tile_pool(name="ep", bufs=2, space="PSUM") as ep,
            tc.tile_pool(name="pb", bufs=1, space="PSUM") as pb,
        ):
            gA = persist.tile([C, D + 1], F32)
            nc.scalar.dma_start(gA[:], gath1[0:C, :])
            gB = persist.tile([C, D + 1], F32)
            nc.scalar.dma_start(gB[:], gath1[C:2 * C, :])

            # winner-take-all merge of the pair's critical instances
            wA = persist.tile([C, 1], F32)
            nc.vector.tensor_tensor(wA[:], gA[:, D:D + 1], gB[:, D:D + 1],
                                    ALU.is_ge)
            md = persist.tile([C, D], F32)
            nc.vector.tensor_tensor(md[:], gA[:, 0:D], gB[:, 0:D],
                                    ALU.subtract)
            ms = persist.tile([C, D], F32)
            nc.vector.tensor_scalar_mul(ms[:], md[:], wA[:])
            m_win = persist.tile([C, D], F32)
            nc.vector.tensor_tensor(m_win[:], ms[:], gB[:, 0:D], ALU.add)

            # q_win = q_fn(m_win)
            mT = persist.tile([128, DB, C], BF16)
            for db in range(DB):
                ptm = pt2.tile([128, C], F32, tag="ptm", name="ptm")
                nc.tensor.transpose(ptm[:],
                                    m_win[:, db * 128:(db + 1) * 128],
                                    identf[0:2, 0:2])
                nc.vector.tensor_copy(mT[:, db, :], ptm[:])
            pzm = pb.tile([128, C], F32, name="pzm")
            for db in range(DB):
                nc.tensor.matmul(pzm[:], w_q1_sb[:, db, :], mT[:, db, :],
                                 start=(db == 0), stop=(db == DB - 1))
            zm = persist.tile([128, C], BF16)
            nc.scalar.activation(zm[:], pzm[:], AF.Relu,
                                 bias=bias_sb[:, DB:DB + 1])
            pqc = pb.tile([128, C], F32, name="pqc")
            nc.tensor.matmul(pqc[:], w_q2_sb[:], zm[:], start=True,
                             stop=True)
            q_win = persist.tile([128, C], BF16)
            nc.scalar.activation(q_win[:], pqc[:], AF.Tanh,
                                 bias=bias_sb[:, DB + 1:DB + 2])

            # e = exp(Q @ q_win / sqrt(qd)) in natural layout
            for cb in range(NCH):
                pe_ = ep.tile([128, BPC, C], F32, tag="e", name="e")
                for b in range(BPC):
                    nc.tensor.matmul(
                        pe_[:, b, :],
                        qt[:, cb, b * 128:(b + 1) * 128],
                        q_win[:], start=True, stop=True)
                nc.scalar.activation(
                    e_nat[:, cb * BPC:(cb + 1) * BPC, :], pe_[:],
                    AF.Exp, scale=inv_sqrt_q)

            # numerator: e^T @ h
            pnum = pb.tile([C, D], F32, name="pnum")
            for nb in range(NB):
                nc.tensor.matmul(pnum[:], e_nat[:, nb, :], h_nat[:, nb, :],
                                 start=(nb == 0), stop=(nb == NB - 1))

            # denominator: cross-instance then cross-partition sum
            denp = persist.tile([128, C], F32)
            nc.vector.reduce_sum(denp[:],
                                 e_nat[:].rearrange("p nb c -> p c nb"),
                                 axis=mybir.AxisListType.X)
            pdt = pb.tile([C, 128], F32, name="pdt")
            nc.tensor.transpose(pdt[:], denp[:], identf[:])
            den = persist.tile([C, 1], F32)
            nc.vector.reduce_sum(den[:], pdt[:], axis=mybir.AxisListType.X)

            num = persist.tile([C, D], F32)
            nc.scalar.copy(num[:], pnum[:])
            pay2 = dram.tile([C, D + 1], F32)
            nc.scalar.dma_start(pay2[:, 0:D], num[:])
            nc.scalar.dma_start(pay2[:, D:D + 1], den[:])
            red2 = dram.tile([C, D + 1], F32)
            nc.gpsimd.collective_compute(
                "AllReduce", ALU.add, replica_groups=groups,
                ins=[pay2[:].opt()], outs=[red2[:].opt()])
            num_s = persist.tile([C, D], F32)
            nc.scalar.dma_start(num_s[:], red2[:, 0:D])
            den_s = persist.tile([C, 1], F32)
            nc.scalar.dma_start(den_s[:], red2[:, D:D + 1])

            recip = persist.tile([C, 1], F32)
            nc.vector.reciprocal(recip[:], den_s[:])
            out_sb = persist.tile([C, D], F32)
            nc.vector.tensor_scalar_mul(out_sb[:], num_s[:], recip[:])
            nc.sync.dma_start(out_d[:], out_sb[:])

    nc.compile()
    return nc


def _make_in_maps(inputs, n_cores=N_CORES, N_loc=N_LOC):
    x = np.asarray(inputs["x"], dtype=np.float32)
    B = x.shape[0]
    D = int(np.asarray(inputs["W_enc"]).shape[1])
    DB = D // 128

    def bf(a):
        return np.ascontiguousarray(np.asarray(a, np.float32).astype(bfdt))

    def blk(a, last):
        # [K, M] -> [128, K//128, M] (partition-major i-block packing)
        a = np.asarray(a, np.float32)
        return np.ascontiguousarray(
            a.reshape(-1, 128, last).transpose(1, 0, 2).astype(bfdt))

    b_enc = np.asarray(inputs["b_enc"], np.float32)
    b_q1 = np.asarray(inputs["b_q1"], np.float32)
    b_q2 = np.asarray(inputs["b_q2"], np.float32)
    bias = np.zeros((128, DB + 2), np.float32)
    bias[:, 0:DB] = b_enc.reshape(DB, 128).T
    bias[:, DB] = b_q1
    bias[:, DB + 1] = b_q2

    shared = {
        "w_enc": blk(inputs["W_enc"], D),
        "w_i": blk(inputs["W_i"], 2),
        "w_q1": blk(inputs["W_q1"], 128),
        "w_q2": bf(inputs["W_q2"]),
        "bias": bias,
        "b_i": bf(np.asarray(inputs["b_i"], np.float32).reshape(1, -1)),
        "identb": np.eye(128, dtype=np.float32).astype(bfdt),
        "identf": np.eye(128, dtype=np.float32),
    }
    xb = x.astype(bfdt)
    in_maps = []
    for core in range(n_cores):
        bag = core // 2
        half = core % 2
        xts = np.ascontiguousarray(
            xb[bag % B, half * N_loc:(half + 1) * N_loc, :].T)
        in_maps.append({"xt": xts, **shared})
    return in_maps


def kernel(**inputs) -> np.ndarray:
    from concourse.bass_utils import run_bass_kernel_spmd

    if "nc" not in _cache:
        _cache["nc"] = _build_kernel()
    nc = _cache["nc"]
    in_maps = _make_in_maps(inputs)
    res = run_bass_kernel_spmd(nc, in_maps, core_ids=list(range(N_CORES)))
    out = np.stack([res.results[2 * b]["out"] for b in range(B_BAGS)])
    return out.astype(np.float32)


# revision 10
# speedup vs baseline: 1.0117x; 1.0117x over previous
"""DSMIL forward pass on 8 Trainium2 NeuronCores (Bass/Tile), bf16 compute.

Sharding: data-parallel over bags, each bag split across a core pair
(core 2b: instances [0:4096) of bag b, core 2b+1: [4096:8192)). Two tiny
pair-local collectives (critical-instance exchange + softmax partial
reduction) keep it a single NEFF launch.

Key implementation choices (vs the fp32r baseline):
  - x is transposed and cast to bf16 on the HOST: the kernel streams
    xT i-block tiles straight into matmuls (no on-chip x transposes,
    half the HBM traffic).
  - all matmuls run in bf16 (fp32 PSUM accumulation); the class-score
    path stays fp32 from PSUM onward so the per-class argmax is stable.
  - classes are computed in NATURAL [n, c] layout directly
    (lhsT = h^T block, rhs = W_i block), so the per-class max reduction
    runs on all 128 DVE lanes instead of 2.
  - weights/constants load via one consolidated DMA each, spread over
    the gpsimd/scalar/vector queues; x chunks own the sync queue.
  - dummy PE transposes at kernel start pre-warm the HAM clock gate
    while the first DMAs land; another batch keeps it warm through the
    AllGather wait.
  - the pair exchanges (m_feats, max) and computes q_fn(m) AFTER the
    winner select; the Q = q_fn(h) pass is emitted around the exchange
    so the collective latency is hidden under real work.
"""
import numpy as np
import ml_dtypes
from contextlib import ExitStack

import concourse.bacc as bacc
import concourse.tile as tile
import concourse.mybir as mybir

F32 = mybir.dt.float32
BF16 = mybir.dt.bfloat16
AF = mybir.ActivationFunctionType
ALU = mybir.AluOpType
bfdt = ml_dtypes.bfloat16

N_CORES = 8
B_BAGS = 4
N_FULL = 8192
N_LOC = N_FULL // 2

_cache = {}


def _build_kernel(n_cores=N_CORES, N_loc=N_LOC, I=1024, D=512, QD=128,
                  C=2, CHUNK=512, N_WARM0=30, N_WARM=64):
    NB = N_loc // 128          # n-blocks (32)
    NCH = N_loc // CHUNK       # chunks (8)
    BPC = CHUNK // 128         # n-blocks per chunk (4)
    IB = I // 128              # i-blocks (8)
    DB = D // 128              # d-blocks (4)
    assert QD == 128 and C == 2
    inv_sqrt_q = 1.0 / float(np.sqrt(QD))

    nc = bacc.Bacc("TRN2", target_bir_lowering=False, debug=False,
                   num_devices=n_cores)

    xt_d = nc.dram_tensor("xt", [N_loc // CHUNK, 128, I // 128, CHUNK],
                          BF16, kind="ExternalInput")
    w_enc = nc.dram_tensor("w_enc", [128, IB, D], BF16, kind="ExternalInput")
    w_i = nc.dram_tensor("w_i", [128, DB, C], BF16, kind="ExternalInput")
    w_q1 = nc.dram_tensor("w_q1", [128, DB, QD], BF16, kind="ExternalInput")
    w_q2 = nc.dram_tensor("w_q2", [QD, QD], BF16, kind="ExternalInput")
    bias_d = nc.dram_tensor("bias", [128, DB + 2], F32, kind="ExternalInput")
    identb_d = nc.dram_tensor("identb", [128, 128], BF16,
                              kind="ExternalInput")
    identf_d = nc.dram_tensor("identf", [128, 128], F32,
                              kind="ExternalInput")
    out_d = nc.dram_tensor("out", [C, D], F32, kind="ExternalOutput")

    groups = [[i, i + 1] for i in range(0, n_cores, 2)]

    with tile.TileContext(nc) as tc, ExitStack() as ctx:
        persist = ctx.enter_context(tc.tile_pool(name="persist", bufs=1))
        dram = ctx.enter_context(tc.tile_pool(name="dram", bufs=1,
                                              space="DRAM"))

        # ---- scratch consts on the (idle) vector queue ----
        scrap = persist.tile([128, 128], BF16)
        nc.vector.memset(scrap[:], 0.0)
        ones_t = persist.tile([1, 128], F32)
        nc.vector.memset(ones_t[:], 1.0)
        warm_in = dram.tile([1, 2], F32)
        nc.scalar.dma_start(warm_in[:], identf_d[0:1, 0:2])
        warm_in2 = dram.tile([1, 2], F32)
        nc.scalar.dma_start(warm_in2[:], identf_d[0:1, 0:2])

        # ---- consolidated weight loads (gpsimd queue) ----
        w_enc_sb = persist.tile([128, IB, D], BF16)
        nc.gpsimd.dma_start(w_enc_sb[:], w_enc[:])
        w_q1_sb = persist.tile([128, DB, QD], BF16)
        nc.gpsimd.dma_start(w_q1_sb[:], w_q1[:])
        w_i_sb = persist.tile([128, DB, C], BF16)
        nc.gpsimd.dma_start(w_i_sb[:], w_i[:])
        w_q2_sb = persist.tile([QD, QD], BF16)
        nc.gpsimd.dma_start(w_q2_sb[:], w_q2[:])

        # warm both collective channels (fires once weights are queued)
        warm_out = dram.tile([2, 2], F32)
        nc.gpsimd.collective_compute(
            "AllGather", ALU.bypass, replica_groups=groups,
            ins=[warm_in[:].opt()], outs=[warm_out[:].opt()])
        warm_out2 = dram.tile([1, 2], F32)
        nc.gpsimd.collective_compute(
            "AllReduce", ALU.add, replica_groups=groups,
            ins=[warm_in2[:].opt()], outs=[warm_out2[:].opt()])

        # ---- small consts (scalar queue) ----
        identb = persist.tile([128, 128], BF16)
        nc.scalar.dma_start(identb[:], identb_d[:])
        identf = persist.tile([128, 128], F32)
        nc.scalar.dma_start(identf[:], identf_d[:])
        bias_sb = persist.tile([128, DB + 2], F32)
        nc.scalar.dma_start(bias_sb[:], bias_d[:])

        # ---- persistent activations ----
        ht_all = persist.tile([128, NCH, DB, CHUNK], BF16)   # h^T
        h_nat = persist.tile([128, NB, D], BF16)             # h natural
        qt = persist.tile([128, NCH, CHUNK], BF16)           # Q^T
        cls_nat = persist.tile([128, NB, C], F32)
        oh = persist.tile([128, NB, C], BF16)
        e_nat = persist.tile([128, NB, C], BF16)

        # ================= phase A: encoder + classes + h transposes ====
        with (
            tc.tile_pool(name="xload", bufs=2) as xload,
            tc.tile_pool(name="hp", bufs=2, space="PSUM") as hp,
            tc.tile_pool(name="tp", bufs=1, space="PSUM") as tp,
            tc.tile_pool(name="cp", bufs=2, space="PSUM") as cp,
        ):
            # pre-warm the PE clock gate while the first DMAs land
            pw = tp.tile([128, D], BF16, tag="t0", name="t")
            for k in range(N_WARM0):
                nc.tensor.transpose(pw[:, 0:128], scrap[:], scrap[:])

            for cb in range(NCH):
                n0 = cb * CHUNK
                xt_c = xload.tile([128, IB, CHUNK], BF16, tag="x", name="x")
                nc.sync.dma_start(xt_c[:], xt_d[cb])

                # h^T = relu(W_enc^T @ xT) per d-block
                for db in range(DB):
                    ph = hp.tile([128, CHUNK], F32, tag="h", name="h")
                    for ib in range(IB):
                        nc.tensor.matmul(
                            ph[:],
                            w_enc_sb[:, ib, db * 128:(db + 1) * 128],
                            xt_c[:, ib, :],
                            start=(ib == 0), stop=(ib == IB - 1))
                    nc.scalar.activation(ht_all[:, cb, db, :], ph[:],
                                         AF.Relu,
                                         bias=bias_sb[:, db:db + 1])

                # h natural layout: 4 transposes per n-block into one tile
                for b in range(BPC):
                    nb = cb * BPC + b
                    pt = tp.tile([128, D], BF16, tag=f"t{b}", name="t")
                    for db in range(DB):
                        nc.tensor.transpose(
                            pt[:, db * 128:(db + 1) * 128],
                            ht_all[:, cb, db, b * 128:(b + 1) * 128],
                            identb[:])
                    nc.vector.tensor_copy(h_nat[:, nb, :], pt[:])

                # classes in natural layout: lhsT = h^T block, rhs = W_i
                pc = cp.tile([128, BPC, C], F32, tag="c", name="c")
                for b in range(BPC):
                    for db in range(DB):
                        nc.tensor.matmul(
                            pc[:, b, :],
                            ht_all[:, cb, db, b * 128:(b + 1) * 128],
                            w_i_sb[:, db, :],
                            start=(db == 0), stop=(db == DB - 1))
                nc.vector.tensor_copy(
                    cls_nat[:, cb * BPC:(cb + 1) * BPC, :], pc[:])

        # ====== exchange prep + Q-pass (hides the AllGather) ===========
        with (
            tc.tile_pool(name="zp", bufs=2, space="PSUM") as zp,
            tc.tile_pool(name="qp", bufs=1, space="PSUM") as qp,
            tc.tile_pool(name="pa", bufs=1, space="PSUM") as pa,
            tc.tile_pool(name="zs", bufs=2) as zs,
        ):
            def q_pass(cb):
                pz = zp.tile([128, CHUNK], F32, tag="z", name="z")
                for db in range(DB):
                    nc.tensor.matmul(pz[:], w_q1_sb[:, db, :],
                                     ht_all[:, cb, db, :],
                                     start=(db == 0), stop=(db == DB - 1))
                zt = zs.tile([128, CHUNK], BF16, tag="zt", name="zt")
                nc.vector.tensor_scalar(zt[:], pz[:],
                                        bias_sb[:, DB:DB + 1], 0.0,
                                        ALU.add, ALU.max)
                pq = qp.tile([128, CHUNK], F32, tag="q", name="q")
                nc.tensor.matmul(pq[:], w_q2_sb[:], zt[:], start=True,
                                 stop=True)
                nc.scalar.activation(qt[:, cb, :], pq[:], AF.Tanh,
                                     bias=bias_sb[:, DB + 1:DB + 2])

            # per-class max across all instances
            rmax = persist.tile([128, C], F32)
            nc.vector.reduce_max(rmax[:],
                                 cls_nat[:].rearrange("p nb c -> p c nb"),
                                 axis=mybir.AxisListType.X)
            q_pass(0)
            pay_sb = persist.tile([128, 2 * C], F32)
            pmv = pa.tile([C, 128], F32, tag="s", name="s", bufs=3)
            nc.tensor.transpose(pmv[:], rmax[:], identf[:])
            mval_c = persist.tile([C, 1], F32)
            nc.vector.reduce_max(mval_c[:], pmv[:],
                                 axis=mybir.AxisListType.X)
            pmr = pa.tile([1, C], F32, tag="s", name="s", bufs=3)
            nc.tensor.transpose(pmr[:], mval_c[:], identf[0:2, 0:2])
            mval_f = persist.tile([1, C], F32)
            nc.vector.tensor_copy(mval_f[:], pmr[:])
            pmb = pa.tile([128, C], F32, tag="s", name="s", bufs=3)
            nc.tensor.matmul(pmb[:], ones_t[:], mval_f[:],
                             start=True, stop=True)
            nc.vector.tensor_copy(pay_sb[:, C:2 * C], pmb[:])
            q_pass(1)

            for nb in range(NB):
                nc.vector.tensor_tensor(oh[:, nb, :], cls_nat[:, nb, :],
                                        pay_sb[:, C:2 * C], ALU.is_equal)

            # m = onehot^T @ h  (critical instance features)
            pmf = pa.tile([C, D], F32, name="pmf")
            for nb in range(NB):
                nc.tensor.matmul(pmf[:], oh[:, nb, :], h_nat[:, nb, :],
                                 start=(nb == 0), stop=(nb == NB - 1))
            q_pass(2)

            # q_cand = q_fn(m); goes into the exchange payload
            m_sb = persist.tile([C, D], F32)
            nc.scalar.copy(m_sb[:], pmf[:])
            mT = persist.tile([128, DB, C], BF16)
            for db in range(DB):
                ptm = pa.tile([128, C], F32, tag="s", name="s", bufs=3)
                nc.tensor.transpose(ptm[:],
                                    m_sb[:, db * 128:(db + 1) * 128],
                                    identf[0:2, 0:2])
                nc.vector.tensor_copy(mT[:, db, :], ptm[:])
            pzm = pa.tile([128, C], F32, tag="s", name="s", bufs=3)
            for db in range(DB):
                nc.tensor.matmul(pzm[:], w_q1_sb[:, db, :], mT[:, db, :],
                                 start=(db == 0), stop=(db == DB - 1))
            zm = persist.tile([128, C], BF16)
            nc.scalar.activation(zm[:], pzm[:], AF.Relu,
                                 bias=bias_sb[:, DB:DB + 1])
            pqc = pa.tile([128, C], F32, tag="s", name="s", bufs=3)
            nc.tensor.matmul(pqc[:], w_q2_sb[:], zm[:], start=True,
                             stop=True)
            nc.scalar.activation(pay_sb[:, 0:C], pqc[:], AF.Tanh,
                                 bias=bias_sb[:, DB + 1:DB + 2])

            pay1 = dram.tile([128, 2 * C], F32)
            nc.scalar.dma_start(pay1[:], pay_sb[:])
            gath1 = dram.tile([256, 2 * C], F32)
            nc.gpsimd.collective_compute(
                "AllGather", ALU.bypass, replica_groups=groups,
                ins=[pay1[:].opt()], outs=[gath1[:].opt()])

            for cb in range(3, NCH):
                q_pass(cb)

            # keep the PE clock gate warm while waiting on the collective
            pwm = pa.tile([128, 128], BF16, name="pwm")
            for k in range(N_WARM):
                nc.tensor.transpose(pwm[:], identb[:], identb[:])

        # ================= phase B: winner, scores, bag output =========
        with (
            tc.tile_pool(name="ep", bufs=2, space="PSUM") as ep,
            tc.tile_pool(name="pb", bufs=1, space="PSUM") as pb,
        ):
            gA = persist.tile([128, 2 * C], F32)
            nc.scalar.dma_start(gA[:], gath1[0:128, :])
            gB = persist.tile([128, 2 * C], F32)
            nc.scalar.dma_start(gB[:], gath1[128:256, :])

            # winner-take-all merge of the pair's candidate queries
            wA = persist.tile([128, C], F32)
            nc.vector.tensor_tensor(wA[:], gA[:, C:2 * C], gB[:, C:2 * C],
                                    ALU.is_ge)
            md = persist.tile([128, C], F32)
            nc.vector.tensor_tensor(md[:], gA[:, 0:C], gB[:, 0:C],
                                    ALU.subtract)
            ms = persist.tile([128, C], F32)
            nc.vector.tensor_tensor(ms[:], md[:], wA[:], ALU.mult)
            q_win = persist.tile([128, C], BF16)
            nc.vector.tensor_tensor(q_win[:], ms[:], gB[:, 0:C], ALU.add)

            # e = exp(Q @ q_win / sqrt(qd)) in natural layout
            for cb in range(NCH):
                pe_ = ep.tile([128, BPC, C], F32, tag="e", name="e")
                for b in range(BPC):
                    nc.tensor.matmul(
                        pe_[:, b, :],
                        qt[:, cb, b * 128:(b + 1) * 128],
                        q_win[:], start=True, stop=True)
                nc.scalar.activation(
                    e_nat[:, cb * BPC:(cb + 1) * BPC, :], pe_[:],
                    AF.Exp, scale=inv_sqrt_q)

            # numerator: e^T @ h
            pnum = pb.tile([C, D], F32, name="pnum")
            for nb in range(NB):
                nc.tensor.matmul(pnum[:], e_nat[:, nb, :], h_nat[:, nb, :],
                                 start=(nb == 0), stop=(nb == NB - 1))

            # denominator: cross-instance then cross-partition sum
            denp = persist.tile([128, C], F32)
            nc.vector.reduce_sum(denp[:],
                                 e_nat[:].rearrange("p nb c -> p c nb"),
                                 axis=mybir.AxisListType.X)
            pdt = pb.tile([C, 128], F32, name="pdt")
            nc.tensor.transpose(pdt[:], denp[:], identf[:])
            den = persist.tile([C, 1], F32)
            nc.vector.reduce_sum(den[:], pdt[:], axis=mybir.AxisListType.X)

            num = persist.tile([C, D], F32)
            nc.scalar.copy(num[:], pnum[:])
            pay2 = dram.tile([C, D + 1], F32)
            nc.scalar.dma_start(pay2[:, 0:D], num[:])
            nc.scalar.dma_start(pay2[:, D:D + 1], den[:])
            red2 = dram.tile([C, D + 1], F32)
            nc.gpsimd.collective_compute(
                "AllReduce", ALU.add, replica_groups=groups,
                ins=[pay2[:].opt()], outs=[red2[:].opt()])
            num_s = persist.tile([C, D], F32)
            nc.scalar.dma_start(num_s[:], red2[:, 0:D])
            den_s = persist.tile([C, 1], F32)
            nc.scalar.dma_start(den_s[:], red2[:, D:D + 1])

            recip = persist.tile([C, 1], F32)
            nc.vector.reciprocal(recip[:], den_s[:])
            out_sb = persist.tile([C, D], F32)
            nc.vector.tensor_scalar_mul(out_sb[:], num_s[:], recip[:])
            nc.sync.dma_start(out_d[:], out_sb[:])

    nc.compile()
    return nc


def _make_in_maps(inputs, n_cores=N_CORES, N_loc=N_LOC):
    x = np.asarray(inputs["x"], dtype=np.float32)
    B = x.shape[0]
    D = int(np.asarray(inputs["W_enc"]).shape[1])
    DB = D // 128

    def bf(a):
        return np.ascontiguousarray(np.asarray(a, np.float32).astype(bfdt))

    def blk(a, last):
        # [K, M] -> [128, K//128, M] (partition-major i-block packing)
        a = np.asarray(a, np.float32)
        return np.ascontiguousarray(
            a.reshape(-1, 128, last).transpose(1, 0, 2).astype(bfdt))

    b_enc = np.asarray(inputs["b_enc"], np.float32)
    b_q1 = np.asarray(inputs["b_q1"], np.float32)
    b_q2 = np.asarray(inputs["b_q2"], np.float32)
    bias = np.zeros((128, DB + 2), np.float32)
    bias[:, 0:DB] = b_enc.reshape(DB, 128).T
    bias[:, DB] = b_q1
    bias[:, DB + 1] = b_q2

    shared = {
        "w_enc": blk(inputs["W_enc"], D),
        "w_i": blk(inputs["W_i"], 2),
        "w_q1": blk(inputs["W_q1"], 128),
        "w_q2": bf(inputs["W_q2"]),
        "bias": bias,
        "identb": np.eye(128, dtype=np.float32).astype(bfdt),
        "identf": np.eye(128, dtype=np.float32),
    }
    xb = x.astype(bfdt)
    NCH = N_loc // 512
    in_maps = []
    for core in range(n_cores):
        bag = core // 2
        half = core % 2
        xh = xb[bag % B, half * N_loc:(half + 1) * N_loc, :]
        # chunk-major: [NCH, 128(p), IB, 512(n)] with 8KB contiguous runs
        xts = np.ascontiguousarray(
            xh.reshape(NCH, 512, -1, 128).transpose(0, 3, 2, 1))
        in_maps.append({"xt": xts, **shared})
    return in_maps


def kernel(**inputs) -> np.ndarray:
    from concourse.bass_utils import run_bass_kernel_spmd

    if "nc" not in _cache:
        _cache["nc"] = _build_kernel()
    nc = _cache["nc"]
    in_maps = _make_in_maps(inputs)
    res = run_bass_kernel_spmd(nc, in_maps, core_ids=list(range(N_CORES)))
    out = np.stack([res.results[2 * b]["out"] for b in range(B_BAGS)])
    return out.astype(np.float32)


# revision 12
# speedup vs baseline: 1.0133x; 1.0016x over previous
"""DSMIL forward pass on 8 Trainium2 NeuronCores (Bass/Tile), bf16 compute.

Sharding: data-parallel over bags, each bag split across a core pair
(core 2b: instances [0:4096) of bag b, core 2b+1: [4096:8192)). Two tiny
pair-local collectives (critical-instance exchange + softmax partial
reduction) keep it a single NEFF launch.

Key implementation choices (vs the fp32r baseline):
  - x is transposed and cast to bf16 on the HOST: the kernel streams
    xT i-block tiles straight into matmuls (no on-chip x transposes,
    half the HBM traffic).
  - all matmuls run in bf16 (fp32 PSUM accumulation); the class-score
    path stays fp32 from PSUM onward so the per-class argmax is stable.
  - classes are computed in NATURAL [n, c] layout directly
    (lhsT = h^T block, rhs = W_i block), so the per-class max reduction
    runs on all 128 DVE lanes instead of 2.
  - weights/constants load via one consolidated DMA each, spread over
    the gpsimd/scalar/vector queues; x chunks own the sync queue.
  - dummy PE transposes at kernel start pre-warm the HAM clock gate
    while the first DMAs land; another batch keeps it warm through the
    AllGather wait.
  - the pair exchanges (m_feats, max) and computes q_fn(m) AFTER the
    winner select; the Q = q_fn(h) pass is emitted around the exchange
    so the collective latency is hidden under real work.
"""
import numpy as np
import ml_dtypes
from contextlib import ExitStack

import concourse.bacc as bacc
import concourse.tile as tile
import concourse.mybir as mybir

F32 = mybir.dt.float32
BF16 = mybir.dt.bfloat16
AF = mybir.ActivationFunctionType
ALU = mybir.AluOpType
bfdt = ml_dtypes.bfloat16

N_CORES = 8
B_BAGS = 4
N_FULL = 8192
N_LOC = N_FULL // 2

_cache = {}


def _build_kernel(n_cores=N_CORES, N_loc=N_LOC, I=1024, D=512, QD=128,
                  C=2, CHUNK=512, N_WARM0=30, N_WARM=16):
    NB = N_loc // 128          # n-blocks (32)
    NCH = N_loc // CHUNK       # chunks (8)
    BPC = CHUNK // 128         # n-blocks per chunk (4)
    IB = I // 128              # i-blocks (8)
    DB = D // 128              # d-blocks (4)
    assert QD == 128 and C == 2
    inv_sqrt_q = 1.0 / float(np.sqrt(QD))

    nc = bacc.Bacc("TRN2", target_bir_lowering=False, debug=False,
                   num_devices=n_cores)

    xt_d = nc.dram_tensor("xt", [N_loc // CHUNK, 128, I // 128, CHUNK],
                          BF16, kind="ExternalInput")
    w_enc = nc.dram_tensor("w_enc", [128, IB, D], BF16, kind="ExternalInput")
    w_i = nc.dram_tensor("w_i", [128, DB, C], BF16, kind="ExternalInput")
    w_q1 = nc.dram_tensor("w_q1", [128, DB, QD], BF16, kind="ExternalInput")
    w_q2 = nc.dram_tensor("w_q2", [QD, QD], BF16, kind="ExternalInput")
    bias_d = nc.dram_tensor("bias", [128, DB + 2], F32, kind="ExternalInput")
    identb_d = nc.dram_tensor("identb", [128, 128], BF16,
                              kind="ExternalInput")
    identf_d = nc.dram_tensor("identf", [128, 128], F32,
                              kind="ExternalInput")
    out_d = nc.dram_tensor("out", [C, D], F32, kind="ExternalOutput")

    groups = [[i, i + 1] for i in range(0, n_cores, 2)]

    with tile.TileContext(nc) as tc, ExitStack() as ctx:
        persist = ctx.enter_context(tc.tile_pool(name="persist", bufs=1))
        dram = ctx.enter_context(tc.tile_pool(name="dram", bufs=1,
                                              space="DRAM"))

        # ---- scratch consts on the (idle) vector queue ----
        scrap = persist.tile([128, 128], BF16)
        nc.vector.memset(scrap[:], 0.0)
        ones_t = persist.tile([1, 128], F32)
        nc.vector.memset(ones_t[:], 1.0)
        warm_in = dram.tile([1, 2], F32)
        nc.scalar.dma_start(warm_in[:], identf_d[0:1, 0:2])
        warm_in2 = dram.tile([1, 2], F32)
        nc.scalar.dma_start(warm_in2[:], identf_d[0:1, 0:2])

        # ---- consolidated weight loads (gpsimd queue) ----
        w_enc_sb = persist.tile([128, IB, D], BF16)
        nc.gpsimd.dma_start(w_enc_sb[:], w_enc[:])
        w_q1_sb = persist.tile([128, DB, QD], BF16)
        nc.gpsimd.dma_start(w_q1_sb[:], w_q1[:])
        w_i_sb = persist.tile([128, DB, C], BF16)
        nc.gpsimd.dma_start(w_i_sb[:], w_i[:])
        w_q2_sb = persist.tile([QD, QD], BF16)
        nc.gpsimd.dma_start(w_q2_sb[:], w_q2[:])

        # warm both collective channels (fires once weights are queued)
        warm_out = dram.tile([2, 2], F32)
        nc.gpsimd.collective_compute(
            "AllGather", ALU.bypass, replica_groups=groups,
            ins=[warm_in[:].opt()], outs=[warm_out[:].opt()])
        warm_out2 = dram.tile([1, 2], F32)
        nc.gpsimd.collective_compute(
            "AllReduce", ALU.add, replica_groups=groups,
            ins=[warm_in2[:].opt()], outs=[warm_out2[:].opt()])

        # ---- small consts (scalar queue) ----
        identb = persist.tile([128, 128], BF16)
        nc.scalar.dma_start(identb[:], identb_d[:])
        identf = persist.tile([128, 128], F32)
        nc.scalar.dma_start(identf[:], identf_d[:])
        bias_sb = persist.tile([128, DB + 2], F32)
        nc.scalar.dma_start(bias_sb[:], bias_d[:])

        # ---- persistent activations ----
        ht_all = persist.tile([128, NCH, DB, CHUNK], BF16)   # h^T
        h_nat = persist.tile([128, NB, D], BF16)             # h natural
        qt = persist.tile([128, NCH, CHUNK], BF16)           # Q^T
        cls_nat = persist.tile([128, NB, C], F32)
        oh = persist.tile([128, NB, C], BF16)
        e_nat = persist.tile([128, NB, C], BF16)

        # ================= phase A: encoder + classes + h transposes ====
        with (
            tc.tile_pool(name="xload", bufs=2) as xload,
            tc.tile_pool(name="hp", bufs=2, space="PSUM") as hp,
            tc.tile_pool(name="tp", bufs=1, space="PSUM") as tp,
            tc.tile_pool(name="cp", bufs=2, space="PSUM") as cp,
        ):
            # pre-warm the PE clock gate while the first DMAs land
            pw = tp.tile([128, D], BF16, tag="t0", name="t")
            for k in range(N_WARM0):
                nc.tensor.transpose(pw[:, 0:128], scrap[:], scrap[:])

            for cb in range(NCH):
                n0 = cb * CHUNK
                xt_c = xload.tile([128, IB, CHUNK], BF16, tag="x", name="x")
                nc.sync.dma_start(xt_c[:], xt_d[cb])

                # h^T = relu(W_enc^T @ xT) per d-block
                for db in range(DB):
                    ph = hp.tile([128, CHUNK], F32, tag="h", name="h")
                    for ib in range(IB):
                        nc.tensor.matmul(
                            ph[:],
                            w_enc_sb[:, ib, db * 128:(db + 1) * 128],
                            xt_c[:, ib, :],
                            start=(ib == 0), stop=(ib == IB - 1))
                    nc.scalar.activation(ht_all[:, cb, db, :], ph[:],
                                         AF.Relu,
                                         bias=bias_sb[:, db:db + 1])

                # h natural layout: 4 transposes per n-block into one tile
                for b in range(BPC):
                    nb = cb * BPC + b
                    pt = tp.tile([128, D], BF16, tag=f"t{b}", name="t")
                    for db in range(DB):
                        nc.tensor.transpose(
                            pt[:, db * 128:(db + 1) * 128],
                            ht_all[:, cb, db, b * 128:(b + 1) * 128],
                            identb[:])
                    nc.vector.tensor_copy(h_nat[:, nb, :], pt[:])

                # classes in natural layout: lhsT = h^T block, rhs = W_i
                pc = cp.tile([128, BPC, C], F32, tag="c", name="c")
                for b in range(BPC):
                    for db in range(DB):
                        nc.tensor.matmul(
                            pc[:, b, :],
                            ht_all[:, cb, db, b * 128:(b + 1) * 128],
                            w_i_sb[:, db, :],
                            start=(db == 0), stop=(db == DB - 1))
                nc.vector.tensor_copy(
                    cls_nat[:, cb * BPC:(cb + 1) * BPC, :], pc[:])

        # ====== exchange prep + Q-pass (hides the AllGather) ===========
        with (
            tc.tile_pool(name="zp", bufs=2, space="PSUM") as zp,
            tc.tile_pool(name="qp", bufs=1, space="PSUM") as qp,
            tc.tile_pool(name="pa", bufs=1, space="PSUM") as pa,
            tc.tile_pool(name="zs", bufs=2) as zs,
        ):
            def q_pass(cb):
                pz = zp.tile([128, CHUNK], F32, tag="z", name="z")
                for db in range(DB):
                    nc.tensor.matmul(pz[:], w_q1_sb[:, db, :],
                                     ht_all[:, cb, db, :],
                                     start=(db == 0), stop=(db == DB - 1))
                zt = zs.tile([128, CHUNK], BF16, tag="zt", name="zt")
                nc.vector.tensor_scalar(zt[:], pz[:],
                                        bias_sb[:, DB:DB + 1], 0.0,
                                        ALU.add, ALU.max)
                pq = qp.tile([128, CHUNK], F32, tag="q", name="q")
                nc.tensor.matmul(pq[:], w_q2_sb[:], zt[:], start=True,
                                 stop=True)
                nc.scalar.activation(qt[:, cb, :], pq[:], AF.Tanh,
                                     bias=bias_sb[:, DB + 1:DB + 2])

            # per-class max across all instances
            rmax = persist.tile([128, C], F32)
            nc.vector.reduce_max(rmax[:],
                                 cls_nat[:].rearrange("p nb c -> p c nb"),
                                 axis=mybir.AxisListType.X)
            q_pass(0)
            pay_sb = persist.tile([128, 2 * C], F32)
            pmv = pa.tile([C, 128], F32, tag="s", name="s", bufs=3)
            nc.tensor.transpose(pmv[:], rmax[:], identf[:])
            mval_c = persist.tile([C, 1], F32)
            nc.vector.reduce_max(mval_c[:], pmv[:],
                                 axis=mybir.AxisListType.X)
            pmr = pa.tile([1, C], F32, tag="s", name="s", bufs=3)
            nc.tensor.transpose(pmr[:], mval_c[:], identf[0:2, 0:2])
            mval_f = persist.tile([1, C], F32)
            nc.vector.tensor_copy(mval_f[:], pmr[:])
            pmb = pa.tile([128, C], F32, tag="s", name="s", bufs=3)
            nc.tensor.matmul(pmb[:], ones_t[:], mval_f[:],
                             start=True, stop=True)
            nc.vector.tensor_copy(pay_sb[:, C:2 * C], pmb[:])
            q_pass(1)

            for nb in range(NB):
                nc.vector.tensor_tensor(oh[:, nb, :], cls_nat[:, nb, :],
                                        pay_sb[:, C:2 * C], ALU.is_equal)

            # m = onehot^T @ h  (critical instance features)
            pmf = pa.tile([C, D], F32, name="pmf")
            for nb in range(NB):
                nc.tensor.matmul(pmf[:], oh[:, nb, :], h_nat[:, nb, :],
                                 start=(nb == 0), stop=(nb == NB - 1))
            q_pass(2)

            # q_cand = q_fn(m); goes into the exchange payload
            m_sb = persist.tile([C, D], F32)
            nc.scalar.copy(m_sb[:], pmf[:])
            mT = persist.tile([128, DB, C], BF16)
            for db in range(DB):
                ptm = pa.tile([128, C], F32, tag="s", name="s", bufs=3)
                nc.tensor.transpose(ptm[:],
                                    m_sb[:, db * 128:(db + 1) * 128],
                                    identf[0:2, 0:2])
                nc.vector.tensor_copy(mT[:, db, :], ptm[:])
            pzm = pa.tile([128, C], F32, tag="s", name="s", bufs=3)
            for db in range(DB):
                nc.tensor.matmul(pzm[:], w_q1_sb[:, db, :], mT[:, db, :],
                                 start=(db == 0), stop=(db == DB - 1))
            zm = persist.tile([128, C], BF16)
            nc.scalar.activation(zm[:], pzm[:], AF.Relu,
                                 bias=bias_sb[:, DB:DB + 1])
            pqc = pa.tile([128, C], F32, tag="s", name="s", bufs=3)
            nc.tensor.matmul(pqc[:], w_q2_sb[:], zm[:], start=True,
                             stop=True)
            nc.scalar.activation(pay_sb[:, 0:C], pqc[:], AF.Tanh,
                                 bias=bias_sb[:, DB + 1:DB + 2])

            pay1 = dram.tile([128, 2 * C], F32)
            nc.scalar.dma_start(pay1[:], pay_sb[:])
            gath1 = dram.tile([256, 2 * C], F32)
            nc.gpsimd.collective_compute(
                "AllGather", ALU.bypass, replica_groups=groups,
                ins=[pay1[:].opt()], outs=[gath1[:].opt()])

            for cb in range(3, NCH):
                q_pass(cb)

            # keep the PE clock gate warm while waiting on the collective
            pwm = pa.tile([128, 128], BF16, name="pwm")
            for k in range(N_WARM):
                nc.tensor.transpose(pwm[:], identb[:], identb[:])

        # ================= phase B: winner, scores, bag output =========
        with (
            tc.tile_pool(name="ep", bufs=2, space="PSUM") as ep,
            tc.tile_pool(name="pb", bufs=1, space="PSUM") as pb,
        ):
            gAB = persist.tile([128, 2, 2 * C], F32)
            nc.scalar.dma_start(
                gAB[:], gath1[:].rearrange("(two p) c -> p two c", p=128))

            # winner-take-all merge of the pair's candidate queries
            wA = persist.tile([128, C], F32)
            nc.vector.tensor_tensor(wA[:], gAB[:, 0, C:2 * C],
                                    gAB[:, 1, C:2 * C], ALU.is_ge)
            md = persist.tile([128, C], F32)
            nc.vector.tensor_tensor(md[:], gAB[:, 0, 0:C], gAB[:, 1, 0:C],
                                    ALU.subtract)
            ms = persist.tile([128, C], F32)
            nc.vector.tensor_tensor(ms[:], md[:], wA[:], ALU.mult)
            q_win = persist.tile([128, C], BF16)
            nc.vector.tensor_tensor(q_win[:], ms[:], gAB[:, 1, 0:C],
                                    ALU.add)

            # e = exp(Q @ q_win / sqrt(qd)) in natural layout
            for cb in range(NCH):
                pe_ = ep.tile([128, BPC, C], F32, tag="e", name="e")
                for b in range(BPC):
                    nc.tensor.matmul(
                        pe_[:, b, :],
                        qt[:, cb, b * 128:(b + 1) * 128],
                        q_win[:], start=True, stop=True)
                nc.scalar.activation(
                    e_nat[:, cb * BPC:(cb + 1) * BPC, :], pe_[:],
                    AF.Exp, scale=inv_sqrt_q)

            # numerator: e^T @ h
            pnum = pb.tile([C, D], F32, name="pnum")
            for nb in range(NB):
                nc.tensor.matmul(pnum[:], e_nat[:, nb, :], h_nat[:, nb, :],
                                 start=(nb == 0), stop=(nb == NB - 1))

            # denominator: cross-instance then cross-partition sum
            denp = persist.tile([128, C], F32)
            nc.vector.reduce_sum(denp[:],
                                 e_nat[:].rearrange("p nb c -> p c nb"),
                                 axis=mybir.AxisListType.X)
            pdt = pb.tile([C, 128], F32, name="pdt")
            nc.tensor.transpose(pdt[:], denp[:], identf[:])
            den = persist.tile([C, 1], F32)
            nc.vector.reduce_sum(den[:], pdt[:], axis=mybir.AxisListType.X)

            num = persist.tile([C, D], F32)
            nc.scalar.copy(num[:], pnum[:])
            pay2 = dram.tile([C, D + 1], F32)
            nc.scalar.dma_start(pay2[:, 0:D], num[:])
            nc.scalar.dma_start(pay2[:, D:D + 1], den[:])
            red2 = dram.tile([C, D + 1], F32)
            nc.gpsimd.collective_compute(
                "AllReduce", ALU.add, replica_groups=groups,
                ins=[pay2[:].opt()], outs=[red2[:].opt()])
            red_sb = persist.tile([C, D + 1], F32)
            nc.scalar.dma_start(red_sb[:], red2[:])

            recip = persist.tile([C, 1], F32)
            nc.vector.reciprocal(recip[:], red_sb[:, D:D + 1])
            out_sb = persist.tile([C, D], F32)
            nc.vector.tensor_scalar_mul(out_sb[:], red_sb[:, 0:D], recip[:])
            nc.sync.dma_start(out_d[:], out_sb[:])

    nc.compile()
    return nc


def _make_in_maps(inputs, n_cores=N_CORES, N_loc=N_LOC):
    x = np.asarray(inputs["x"], dtype=np.float32)
    B = x.shape[0]
    D = int(np.asarray(inputs["W_enc"]).shape[1])
    DB = D // 128

    def bf(a):
        return np.ascontiguousarray(np.asarray(a, np.float32).astype(bfdt))

    def blk(a, last):
        # [K, M] -> [128, K//128, M] (partition-major i-block packing)
        a = np.asarray(a, np.float32)
        return np.ascontiguousarray(
            a.reshape(-1, 128, last).transpose(1, 0, 2).astype(bfdt))

    b_enc = np.asarray(inputs["b_enc"], np.float32)
    b_q1 = np.asarray(inputs["b_q1"], np.float32)
    b_q2 = np.asarray(inputs["b_q2"], np.float32)
    bias = np.zeros((128, DB + 2), np.float32)
    bias[:, 0:DB] = b_enc.reshape(DB, 128).T
    bias[:, DB] = b_q1
    bias[:, DB + 1] = b_q2

    shared = {
        "w_enc": blk(inputs["W_enc"], D),
        "w_i": blk(inputs["W_i"], 2),
        "w_q1": blk(inputs["W_q1"], 128),
        "w_q2": bf(inputs["W_q2"]),
        "bias": bias,
        "identb": np.eye(128, dtype=np.float32).astype(bfdt),
        "identf": np.eye(128, dtype=np.float32),
    }
    xb = x.astype(bfdt)
    NCH = N_loc // 512
    in_maps = []
    for core in range(n_cores):
        bag = core // 2
        half = core % 2
        xh = xb[bag % B, half * N_loc:(half + 1) * N_loc, :]
        # chunk-major: [NCH, 128(p), IB, 512(n)] with 8KB contiguous runs
        xts = np.ascontiguousarray(
            xh.reshape(NCH, 512, -1, 128).transpose(0, 3, 2, 1))
        in_maps.append({"xt": xts, **shared})
    return in_maps


def kernel(**inputs) -> np.ndarray:
    from concourse.bass_utils import run_bass_kernel_spmd

    if "nc" not in _cache:
        _cache["nc"] = _build_kernel()
    nc = _cache["nc"]
    in_maps = _make_in_maps(inputs)
    res = run_bass_kernel_spmd(nc, in_maps, core_ids=list(range(N_CORES)))
    out = np.stack([res.results[2 * b]["out"] for b in range(B_BAGS)])
    return out.astype(np.float32)
